# revision 19
# baseline (speedup 1.0000x reference)
"""AdaMoLE forward on 8 Trainium2 NeuronCores (Bass/Tile), data-parallel over tokens.

Reference computation (per token n):
  logits = x @ router_w.T + router_b            [N, E]
  gate   = softmax(logits)                      [N, E]
  thr    = sigmoid(x @ thr_w.T + thr_b)/E       [N, 1]
  w      = relu(gate - thr); w /= max(sum(w), eps-guard)
  h      = einsum('nd,erd->ner', x, lora_A)
  out    = einsum('ner,eor->no', h * w[:, :, None], lora_B) * SCALING

Device mapping (per core, NL=1024 tokens):
  phase1: stream xT[d, tok] over 32 d-chunks; accumulate in PSUM:
          hT[er=128, tok] (lhsT=A2 chunks) and lgT[9, tok] (lhsT=[router|thr]T)
  gating: PE-transpose lgT -> [tok, 9]; softmax-free trick:
          w ∝ relu(exp(logits) - sum(exp)*thr)  (common positive scale cancels
          in the normalization), normalize by free-dim sum, transpose back,
          expand e->(e,r) rows via a 0/1 REP matmul -> wexpT[128, tok]
  phase2: whT = hT * wexpT;  out[tok, o] = whT.T @ B2  (B2 pre-scaled by SCALING)

All big operands are host-pre-transposed (and cast to bf16) so every DMA is
contiguous and the PE runs at full bf16 rate. Gating math runs in f32 on-chip;
PSUM accumulation is f32 throughout. Output is f32.
"""

import numpy as np

N_TOKENS = 8192
D_IN = 4096
D_OUT = 4096
E = 8
R = 16
ER = E * R  # 128
SCALING = 32.0 / 16.0
MAX_THRESHOLD = 1.0 / E
N_CORES = 8
NL = N_TOKENS // N_CORES  # tokens per core
DC = D_IN // 128  # 32 d-chunks
TCH = NL // 128  # 8 token chunks per core
OC = D_OUT // 512  # 8 output column chunks

_CACHE = {}


def _build_nc(dbg=False, repeat=1):
    import concourse.mybir as mybir
    import concourse.tile as tile
    from concourse import bacc
    from concourse.masks import make_identity

    f32 = mybir.dt.float32
    bf16 = mybir.dt.bfloat16
    AF = mybir.ActivationFunctionType
    ALU = mybir.AluOpType

    nc = bacc.Bacc("TRN2", target_bir_lowering=False, debug=False)
    dbg_t = {}
    if dbg:
        dbg_t["lg"] = nc.declare_dram_parameter("dbg_lg", [9, NL], f32, isOutput=True)
        dbg_t["h"] = nc.declare_dram_parameter("dbg_h", [128, NL], f32, isOutput=True)
        dbg_t["wn"] = nc.declare_dram_parameter("dbg_wn", [E, NL], f32, isOutput=True)
        dbg_t["wh"] = nc.declare_dram_parameter("dbg_wh", [128, NL], f32, isOutput=True)

    xT = nc.declare_dram_parameter("xT", [DC, 128, NL], bf16, isOutput=False)
    A2 = nc.declare_dram_parameter("A2", [DC, 128, ER], bf16, isOutput=False)
    W9 = nc.declare_dram_parameter("W9", [DC, 128, 16], bf16, isOutput=False)
    B9 = nc.declare_dram_parameter("B9", [1, 16], bf16, isOutput=False)
    B2 = nc.declare_dram_parameter("B2", [ER, D_OUT], bf16, isOutput=False)
    # G[0:8, (e,r)] = delta_{e,e'}; G[8, :] = -1  (expand-and-subtract matrix)
    G9 = nc.declare_dram_parameter("G9", [9, ER], bf16, isOutput=False)
    out = nc.declare_dram_parameter("out", [NL, D_OUT], f32, isOutput=True)

    with tile.TileContext(nc) as tc:
        with (
            tc.tile_pool(name="const", bufs=1) as cpool,
            tc.tile_pool(name="xt", bufs=3) as xpool,
            tc.tile_pool(name="work", bufs=1) as wpool,
            tc.tile_pool(name="osb", bufs=2) as opool,
            tc.tile_pool(name="accps", bufs=2, space="PSUM") as acc_ps,
            tc.tile_pool(name="outps", bufs=4, space="PSUM") as out_ps,
        ):
          for _rep in range(repeat):
            # ---- constants / weights into SBUF ----
            a2_sb = cpool.tile([128, DC, ER], bf16)
            nc.sync.dma_start(out=a2_sb[:], in_=A2.ap().rearrange("a b c -> b a c"))
            w9_sb = cpool.tile([128, DC, 16], bf16)
            nc.sync.dma_start(out=w9_sb[:], in_=W9.ap().rearrange("a b c -> b a c"))
            b9_sb = cpool.tile([1, 16], bf16)
            nc.sync.dma_start(out=b9_sb[:], in_=B9.ap())
            b2_sb = cpool.tile([ER, D_OUT], bf16)
            nc.sync.dma_start(out=b2_sb[:], in_=B2.ap())
            g9_sb = cpool.tile([9, ER], bf16)
            nc.sync.dma_start(out=g9_sb[:], in_=G9.ap())
            ones_sb = cpool.tile([128, 512], bf16)
            nc.vector.memset(ones_sb[:], 1.0)

            # ---- phase 1: accumulate hT[128, NL] and lgT[9, NL] over d-chunks
            h_acc = acc_ps.tile([128, NL], f32, tag="acc")
            lg_acc = acc_ps.tile([128, NL], f32, tag="acc")
            for dc in range(DC):
                xt = xpool.tile([128, NL], bf16)
                nc.sync.dma_start(out=xt[:], in_=xT.ap()[dc])
                for g in range(NL // 512):
                    sl = slice(g * 512, (g + 1) * 512)
                    nc.tensor.matmul(
                        h_acc[:, sl],
                        a2_sb[:, dc, :],
                        xt[:, sl],
                        start=(dc == 0),
                        stop=(dc == DC - 1),
                    )
                    nc.tensor.matmul(
                        lg_acc[0:9, sl],
                        w9_sb[:, dc, 0:9],
                        xt[:, sl],
                        start=(dc == 0),
                        stop=False,
                    )
            # bias row: lg += bias9.T @ ones
            for g in range(NL // 512):
                sl = slice(g * 512, (g + 1) * 512)
                nc.tensor.matmul(
                    lg_acc[0:9, sl],
                    b9_sb[0:1, 0:9],
                    ones_sb[0:1, 0:512],
                    start=False,
                    stop=True,
                )

            # ---- gating, entirely in the [e, tok] domain (no transposes) ----
            # stack9 rows 0:8 = exp(logits) (bf16), row 8 = S*thr*MAX_T
            stack9 = wpool.tile([9, NL], bf16)
            thr_sb = wpool.tile([1, NL], f32)
            nc.scalar.activation(stack9[0:8, :], lg_acc[0:8, :], AF.Exp)
            nc.scalar.activation(thr_sb[:], lg_acc[8:9, :], AF.Sigmoid)
            # S[1, tok] = sum_e exp
            s_ps = acc_ps.tile([128, NL], f32, tag="acc")
            for g in range(NL // 512):
                sl = slice(g * 512, (g + 1) * 512)
                nc.tensor.matmul(
                    s_ps[0:1, sl],
                    ones_sb[0:8, 0:1],
                    stack9[0:8, sl],
                    start=True,
                    stop=True,
                )
            # sthr = (S * MAX_T) * thr  -> stack9 row 8
            nc.vector.scalar_tensor_tensor(
                out=stack9[8:9, :],
                in0=s_ps[0:1, :],
                scalar=MAX_THRESHOLD,
                in1=thr_sb[:],
                op0=ALU.mult,
                op1=ALU.mult,
            )
            # P1[(e,r), tok] = exp_e - sthr  (expand + subtract in one matmul)
            p1_ps = acc_ps.tile([128, NL], f32, tag="acc")
            for g in range(NL // 512):
                sl = slice(g * 512, (g + 1) * 512)
                nc.tensor.matmul(
                    p1_ps[:, sl], g9_sb[:, :], stack9[:, sl],
                    start=True, stop=True,
                )
            # wtexp = relu(P1)  (unnormalized weights, expanded to (e,r) rows)
            wtexp_sb = wpool.tile([128, NL], bf16)
            nc.vector.tensor_scalar_max(wtexp_sb[:], p1_ps[:], 0.0)
            # wsum16[1, tok] = sum over all 128 rows = 16 * sum_e w
            # (the 1/16 is folded into B2's host-side scale)
            ws_ps = acc_ps.tile([128, NL], f32, tag="acc")
            for g in range(NL // 512):
                sl = slice(g * 512, (g + 1) * 512)
                nc.tensor.matmul(
                    ws_ps[0:1, sl], ones_sb[:, 0:1], wtexp_sb[:, sl],
                    start=True, stop=True,
                )
            # guard exact-0 then reciprocal
            rcp_sb = wpool.tile([1, NL], bf16)
            nc.vector.tensor_scalar_max(ws_ps[0:1, :], ws_ps[0:1, :], 1e-30)
            with nc.allow_low_precision(reason="recip rounds to bf16 on write"):
                nc.vector.reciprocal(rcp_sb[:], ws_ps[0:1, :])
            # expand recip to all 128 rows
            rexp_ps = acc_ps.tile([128, NL], f32, tag="acc")
            for g in range(NL // 512):
                sl = slice(g * 512, (g + 1) * 512)
                nc.tensor.matmul(
                    rexp_ps[:, sl], ones_sb[0:1, 0:128], rcp_sb[:, sl],
                    start=True, stop=True,
                )
            # wn = wtexp * recip;  wh = h * wn
            wn_sb = wpool.tile([128, NL], f32)
            nc.vector.tensor_tensor(
                out=wn_sb[:], in0=rexp_ps[:], in1=wtexp_sb[:], op=ALU.mult
            )
            wh_sb = wpool.tile([128, NL], bf16)
            nc.vector.tensor_tensor(
                out=wh_sb[:], in0=h_acc[:], in1=wn_sb[:], op=ALU.mult
            )
            if dbg:
                h_sb = wpool.tile([128, NL], f32)
                nc.scalar.copy(out=h_sb[:], in_=h_acc[:])
                nc.sync.dma_start(out=dbg_t["h"].ap()[:, :], in_=h_sb[:])
                lg_sb = wpool.tile([9, NL], f32)
                nc.scalar.copy(out=lg_sb[:], in_=lg_acc[0:9, :])
                nc.sync.dma_start(out=dbg_t["lg"].ap()[:, :], in_=lg_sb[:])
                nc.sync.dma_start(out=dbg_t["wn"].ap()[:, :], in_=wn_sb[0:E, :])
                wh_f32 = wpool.tile([128, NL], f32)
                nc.vector.tensor_copy(out=wh_f32[:], in_=wh_sb[:])
                nc.sync.dma_start(out=dbg_t["wh"].ap()[:, :], in_=wh_f32[:])

            # ---- phase 2: out[tok, o] = whT.T @ B2 ----
            for t in range(TCH):
                ts = slice(t * 128, (t + 1) * 128)
                o_sb = opool.tile([128, D_OUT], f32, tag="osb")
                for oc in range(OC):
                    osl = slice(oc * 512, (oc + 1) * 512)
                    po = out_ps.tile([128, 512], f32, tag="po")
                    nc.tensor.matmul(
                        po[:],
                        wh_sb[:, ts],
                        b2_sb[:, osl],
                        start=True,
                        stop=True,
                    )
                    if oc % 2 == 0:
                        nc.scalar.copy(out=o_sb[:, osl], in_=po[:])
                    else:
                        nc.vector.tensor_copy(out=o_sb[:, osl], in_=po[:])
                nc.sync.dma_start(out=out.ap()[ts, :], in_=o_sb[:])

    nc.compile()
    return nc


def _make_runner(nc, n_cores=N_CORES):
    import jax
    import numpy as np
    from jax.sharding import Mesh, NamedSharding, PartitionSpec
    from jax.experimental.shard_map import shard_map
    import concourse.mybir as mybir
    from concourse.bass2jax import (
        _bass_exec_p,
        install_neuronx_cc_hook,
        partition_id_tensor,
    )

    install_neuronx_cc_hook()
    partition_name = nc.partition_id_tensor.name if nc.partition_id_tensor else None
    in_names, out_names, out_avals = [], [], []
    for alloc in nc.m.functions[0].allocations:
        if not isinstance(alloc, mybir.MemoryLocationSet):
            continue
        name = alloc.memorylocations[0].name
        if alloc.kind == "ExternalInput":
            if name != partition_name:
                in_names.append(name)
        elif alloc.kind == "ExternalOutput":
            out_names.append(name)
            out_avals.append(
                jax.core.ShapedArray(
                    tuple(alloc.tensor_shape), mybir.dt.np(alloc.dtype)
                )
            )
    n_params = len(in_names)
    n_outs = len(out_avals)
    all_in_names = in_names + out_names + ([partition_name] if partition_name else [])

    def _body(*args):
        operands = list(args)
        if partition_name is not None:
            operands.append(partition_id_tensor())
        outs = _bass_exec_p.bind(
            *operands,
            out_avals=tuple(out_avals),
            in_names=tuple(all_in_names),
            out_names=tuple(out_names),
            lowering_input_output_aliases=(),
            sim_require_finite=True,
            sim_require_nnan=True,
            nc=nc,
        )
        return tuple(outs)

    devices = jax.devices()[:n_cores]
    mesh = Mesh(np.asarray(devices), ("core",))
    sharding = NamedSharding(mesh, PartitionSpec("core"))
    in_specs = (PartitionSpec("core"),) * (n_params + n_outs)
    out_specs = (PartitionSpec("core"),) * n_outs

    fn1 = jax.jit(
        shard_map(
            _body, mesh=mesh, in_specs=in_specs, out_specs=out_specs, check_rep=False
        ),
        donate_argnums=tuple(range(n_params, n_params + n_outs)),
        keep_unused=True,
    )

    def _chain_factory(k):
        def chain(*args):
            ins = list(args[:n_params])
            z = list(args[n_params:])
            for _ in range(k):
                z = list(_body(*ins, *z))
            return tuple(z)

        return jax.jit(
            shard_map(
                chain,
                mesh=mesh,
                in_specs=in_specs,
                out_specs=out_specs,
                check_rep=False,
            ),
            donate_argnums=tuple(range(n_params, n_params + n_outs)),
            keep_unused=True,
        )

    return {
        "fn1": fn1,
        "chain_factory": _chain_factory,
        "in_names": in_names,
        "out_names": out_names,
        "out_avals": out_avals,
        "mesh": mesh,
        "sharding": sharding,
        "n_params": n_params,
    }


def _get_runner():
    if "runner" not in _CACHE:
        nc = _build_nc()
        _CACHE["nc"] = nc
        _CACHE["runner"] = _make_runner(nc)
    return _CACHE["runner"]


def _prep_inputs(inputs, router_w, router_b, thr_w, thr_b, lora_A, lora_B):
    """Host-side staging: transposes + bf16 casts + sharding. Returns dict of
    global (concatenated along axis 0) arrays."""
    import ml_dtypes

    bf16 = ml_dtypes.bfloat16
    x = np.asarray(inputs, dtype=np.float32)
    xT = np.ascontiguousarray(x.T.astype(bf16)).reshape(DC, 128, N_TOKENS)
    # A2[d, (e, r)] with e-major columns
    A2 = (
        np.ascontiguousarray(np.asarray(lora_A, np.float32).reshape(ER, D_IN).T)
        .astype(bf16)
        .reshape(DC, 128, ER)
    )
    # W9[d, 0:8]=router, [d, 8]=thr, padded to 16 cols
    W9 = np.zeros((D_IN, 16), np.float32)
    W9[:, 0:8] = np.asarray(router_w, np.float32).T
    W9[:, 8] = np.asarray(thr_w, np.float32)[0]
    W9 = W9.astype(bf16).reshape(DC, 128, 16)
    B9 = np.zeros((1, 16), np.float32)
    B9[0, 0:8] = np.asarray(router_b, np.float32)
    B9[0, 8] = np.asarray(thr_b, np.float32)[0]
    B9 = B9.astype(bf16)
    # B2[(e, r), o], pre-scaled (the extra 16 undoes wsum16 = 16*wsum)
    B2 = np.ascontiguousarray(
        np.asarray(lora_B, np.float32).transpose(0, 2, 1).reshape(ER, D_OUT)
        * (SCALING * 16.0)
    ).astype(bf16)
    G9m = np.zeros((9, ER), np.float32)
    for e in range(E):
        G9m[e, e * R : (e + 1) * R] = 1.0
    G9m[8, :] = -1.0
    G9m = G9m.astype(bf16)
    per_core = {
        "A2": A2,
        "W9": W9,
        "B9": B9,
        "B2": B2,
        "G9": G9m,
    }
    arrays = {}
    for name in ("A2", "W9", "B9", "B2", "G9"):
        a = per_core[name]
        arrays[name] = np.broadcast_to(
            a, (N_CORES,) + a.shape
        ).reshape((N_CORES * a.shape[0],) + a.shape[1:])
    # xT shards: tokens split along the last axis -> per-core [DC, 128, NL]
    xT_shards = [
        np.ascontiguousarray(xT[:, :, c * NL : (c + 1) * NL]) for c in range(N_CORES)
    ]
    arrays["xT"] = np.concatenate(xT_shards, axis=0)
    return arrays


def kernel(inputs, router_w, router_b, thr_w, thr_b, lora_A, lora_B):
    import jax

    r = _get_runner()
    arrays = _prep_inputs(
        inputs, router_w, router_b, thr_w, thr_b, lora_A, lora_B
    )
    ins = [arrays[name] for name in r["in_names"]]
    zeros = [
        np.zeros((N_CORES * a.shape[0],) + a.shape[1:], a.dtype)
        for a in r["out_avals"]
    ]
    outs = r["fn1"](*ins, *zeros)
    out = np.asarray(jax.block_until_ready(outs[0]))
    return out.reshape(N_TOKENS, D_OUT)


# revision 36
# speedup vs baseline: 2.0022x; 2.0022x over previous
"""AdaMoLE forward on 8 Trainium2 NeuronCores (Bass/Tile), data-parallel over tokens.

Reference computation (per token n):
  logits = x @ router_w.T + router_b            [N, E]
  gate   = softmax(logits)                      [N, E]
  thr    = sigmoid(x @ thr_w.T + thr_b)/E       [N, 1]
  w      = relu(gate - thr); w /= max(sum(w), eps-guard)
  h      = einsum('nd,erd->ner', x, lora_A)
  out    = einsum('ner,eor->no', h * w[:, :, None], lora_B) * SCALING

Device mapping (per core, NL=1024 tokens):
  phase1: stream xT[d, tok] over 32 d-chunks; accumulate in PSUM:
          hT[er=128, tok] (lhsT=A2 chunks) and lgT[9, tok] (lhsT=[router|thr]T)
  gating: PE-transpose lgT -> [tok, 9]; softmax-free trick:
          w ∝ relu(exp(logits) - sum(exp)*thr)  (common positive scale cancels
          in the normalization), normalize by free-dim sum, transpose back,
          expand e->(e,r) rows via a 0/1 REP matmul -> wexpT[128, tok]
  phase2: whT = hT * wexpT;  out[tok, o] = whT.T @ B2  (B2 pre-scaled by SCALING)

All big operands are host-pre-transposed (and cast to bf16) so every DMA is
contiguous and the PE runs at full bf16 rate. Gating math runs in f32 on-chip;
PSUM accumulation is f32 throughout. Output is f32.
"""

import numpy as np

N_TOKENS = 8192
D_IN = 4096
D_OUT = 4096
E = 8
R = 16
ER = E * R  # 128
SCALING = 32.0 / 16.0
MAX_THRESHOLD = 1.0 / E
N_CORES = 8
NL = N_TOKENS // N_CORES  # tokens per core
DC = D_IN // 128  # 32 d-chunks
TCH = NL // 128  # 8 token chunks per core
OC = D_OUT // 512  # 8 output column chunks

_CACHE = {}

# tuning knobs (read at _build_nc time)
COPY_ACT_EVERY = 2  # 1 of every N phase-2 PSUM->SBUF copies goes to ScalarE
A2_DMA_SPLIT = 4  # weight DMA split count (lets phase-1 start earlier)
OUT_BF16 = True  # ship the output as bf16 and upcast on the host


def _build_nc(dbg=False, repeat=1, ablate=frozenset()):
    import concourse.mybir as mybir
    import concourse.tile as tile
    from concourse import bacc
    from concourse.masks import make_identity

    f32 = mybir.dt.float32
    bf16 = mybir.dt.bfloat16
    AF = mybir.ActivationFunctionType
    ALU = mybir.AluOpType

    nc = bacc.Bacc("TRN2", target_bir_lowering=False, debug=False)
    dbg_t = {}
    if dbg:
        dbg_t["lg"] = nc.declare_dram_parameter("dbg_lg", [9, NL], f32, isOutput=True)
        dbg_t["h"] = nc.declare_dram_parameter("dbg_h", [128, NL], f32, isOutput=True)
        dbg_t["wn"] = nc.declare_dram_parameter("dbg_wn", [E, NL], f32, isOutput=True)
        dbg_t["wh"] = nc.declare_dram_parameter("dbg_wh", [128, NL], f32, isOutput=True)

    xT = nc.declare_dram_parameter("xT", [DC, 128, NL], bf16, isOutput=False)
    A2 = nc.declare_dram_parameter("A2", [DC, 128, ER], bf16, isOutput=False)
    W9 = nc.declare_dram_parameter("W9", [DC, 128, 48], bf16, isOutput=False)
    B9 = nc.declare_dram_parameter("B9", [1, 48], bf16, isOutput=False)
    B2 = nc.declare_dram_parameter("B2", [ER, D_OUT], bf16, isOutput=False)
    # G[0:8, (e,r)] = delta_{e,e'}; G[8, :] = -1  (expand-and-subtract matrix)
    G9 = nc.declare_dram_parameter("G9", [9, ER], bf16, isOutput=False)
    out_dt = bf16 if OUT_BF16 else f32
    out = nc.declare_dram_parameter("out", [NL, D_OUT], out_dt, isOutput=True)

    with tile.TileContext(nc) as tc:
        with (
            tc.tile_pool(name="const", bufs=1) as cpool,
            tc.tile_pool(name="xt", bufs=6) as xpool,
            tc.tile_pool(name="work", bufs=1) as wpool,
            tc.tile_pool(name="osb", bufs=2) as opool,
            tc.tile_pool(name="accps", bufs=2, space="PSUM") as acc_ps,
            tc.tile_pool(name="outps", bufs=4, space="PSUM") as out_ps,
        ):
          for _rep in range(repeat):
            # ---- constants / weights into SBUF ----
            a2_sb = cpool.tile([128, DC, ER], bf16)
            for q in range(A2_DMA_SPLIT):
                qs = slice(q * (DC // A2_DMA_SPLIT), (q + 1) * (DC // A2_DMA_SPLIT))
                nc.sync.dma_start(
                    out=a2_sb[:, qs, :],
                    in_=A2.ap()[qs].rearrange("a b c -> b a c"),
                )
            w9_sb = cpool.tile([128, DC, 48], bf16)
            nc.sync.dma_start(out=w9_sb[:], in_=W9.ap().rearrange("a b c -> b a c"))
            b9_sb = cpool.tile([1, 48], bf16)
            nc.sync.dma_start(out=b9_sb[:], in_=B9.ap())
            b2_sb = cpool.tile([ER, D_OUT], bf16)
            nc.sync.dma_start(out=b2_sb[:], in_=B2.ap())
            g9_sb = cpool.tile([9, ER], bf16)
            nc.sync.dma_start(out=g9_sb[:], in_=G9.ap())
            ones_sb = cpool.tile([128, 512], bf16)
            nc.vector.memset(ones_sb[:], 1.0)

            # ---- phase 1: accumulate hT[128, NL] and lgT[9, NL] over d-chunks
            h_acc = acc_ps.tile([128, NL], f32, tag="acc")
            lg_acc = acc_ps.tile([128, NL], f32, tag="acc")
            for dc in range(DC):
                xt = xpool.tile([128, NL], bf16)
                nc.sync.dma_start(out=xt[:], in_=xT.ap()[dc])
                for g in range(NL // 512):
                    sl = slice(g * 512, (g + 1) * 512)
                    nc.tensor.matmul(
                        h_acc[:, sl],
                        a2_sb[:, dc, :],
                        xt[:, sl],
                        start=(dc == 0),
                        stop=(dc == DC - 1),
                    )
                    nc.tensor.matmul(
                        lg_acc[0:33, sl],
                        w9_sb[:, dc, 0:33],
                        xt[:, sl],
                        start=(dc == 0),
                        stop=False,
                    )
            # bias row: lg += bias9.T @ ones
            for g in range(NL // 512):
                sl = slice(g * 512, (g + 1) * 512)
                nc.tensor.matmul(
                    lg_acc[0:33, sl],
                    b9_sb[0:1, 0:33],
                    ones_sb[0:1, 0:512],
                    start=False,
                    stop=True,
                )

            # ---- gating, entirely in the [e, tok] domain (no transposes) ----
            ex_sb = wpool.tile([E, NL], bf16)
            thr_sb = wpool.tile([1, NL], f32)
            nc.scalar.activation(ex_sb[:], lg_acc[0:8, :], AF.Exp)
            nc.scalar.activation(thr_sb[:], lg_acc[32:33, :], AF.Sigmoid)
            # S[1, tok] = sum_e exp
            s_ps = acc_ps.tile([128, NL], f32, tag="acc")
            for g in range(NL // 512):
                sl = slice(g * 512, (g + 1) * 512)
                nc.tensor.matmul(
                    s_ps[0:1, sl],
                    ones_sb[0:8, 0:1],
                    ex_sb[:, sl],
                    start=True,
                    stop=True,
                )
            # nsthr = -(S * MAX_T) * thr
            nsthr_sb = wpool.tile([1, NL], bf16)
            nc.vector.scalar_tensor_tensor(
                out=nsthr_sb[:],
                in0=s_ps[0:1, :],
                scalar=-MAX_THRESHOLD,
                in1=thr_sb[:],
                op0=ALU.mult,
                op1=ALU.mult,
            )
            # P1[(e,r), tok] = exp_e - sthr  (delta-expand exp, add -sthr bcast)
            p1_ps = acc_ps.tile([128, NL], f32, tag="acc")
            for g in range(NL // 512):
                sl = slice(g * 512, (g + 1) * 512)
                nc.tensor.matmul(
                    p1_ps[:, sl], g9_sb[0:8, :], ex_sb[:, sl],
                    start=True, stop=False,
                )
                nc.tensor.matmul(
                    p1_ps[:, sl], ones_sb[0:1, 0:128], nsthr_sb[:, sl],
                    start=False, stop=True,
                )
            # wtexp = relu(P1)  (unnormalized weights, expanded to (e,r) rows)
            wtexp_sb = wpool.tile([128, NL], bf16)
            nc.vector.tensor_scalar_max(wtexp_sb[:], p1_ps[:], 0.0)
            # wsum16[1, tok] = sum over all 128 rows = 16 * sum_e w
            # (the 1/16 is folded into B2's host-side scale)
            ws_ps = acc_ps.tile([128, NL], f32, tag="acc")
            for g in range(NL // 512):
                sl = slice(g * 512, (g + 1) * 512)
                nc.tensor.matmul(
                    ws_ps[0:1, sl], ones_sb[:, 0:1], wtexp_sb[:, sl],
                    start=True, stop=True,
                )
            # guard exact-0 then reciprocal
            rcp_sb = wpool.tile([1, NL], bf16)
            nc.vector.tensor_scalar_max(ws_ps[0:1, :], ws_ps[0:1, :], 1e-30)
            with nc.allow_low_precision(reason="recip rounds to bf16 on write"):
                nc.vector.reciprocal(rcp_sb[:], ws_ps[0:1, :])
            # expand recip to all 128 rows
            rexp_ps = acc_ps.tile([128, NL], f32, tag="acc")
            for g in range(NL // 512):
                sl = slice(g * 512, (g + 1) * 512)
                nc.tensor.matmul(
                    rexp_ps[:, sl], ones_sb[0:1, 0:128], rcp_sb[:, sl],
                    start=True, stop=True,
                )
            # wn = wtexp * recip;  wh = h * wn
            wn_sb = wpool.tile([128, NL], f32)
            nc.vector.tensor_tensor(
                out=wn_sb[:], in0=rexp_ps[:], in1=wtexp_sb[:], op=ALU.mult
            )
            wh_sb = wpool.tile([128, NL], bf16)
            nc.vector.tensor_tensor(
                out=wh_sb[:], in0=h_acc[:], in1=wn_sb[:], op=ALU.mult
            )
            if dbg:
                h_sb = wpool.tile([128, NL], f32)
                nc.scalar.copy(out=h_sb[:], in_=h_acc[:])
                nc.sync.dma_start(out=dbg_t["h"].ap()[:, :], in_=h_sb[:])
                lg_sb = wpool.tile([9, NL], f32)
                nc.scalar.copy(out=lg_sb[:], in_=lg_acc[0:9, :])
                nc.sync.dma_start(out=dbg_t["lg"].ap()[:, :], in_=lg_sb[:])
                nc.sync.dma_start(out=dbg_t["wn"].ap()[:, :], in_=wn_sb[0:E, :])
                wh_f32 = wpool.tile([128, NL], f32)
                nc.vector.tensor_copy(out=wh_f32[:], in_=wh_sb[:])
                nc.sync.dma_start(out=dbg_t["wh"].ap()[:, :], in_=wh_f32[:])

            # ---- phase 2: out[tok, o] = whT.T @ B2 ----
            for t in range(TCH):
                ts = slice(t * 128, (t + 1) * 128)
                o_sb = opool.tile([128, D_OUT], out_dt, tag="osb")
                if "p2mm" in ablate:
                    nc.scalar.memzero(o_sb[:])
                else:
                    for oc in range(OC):
                        osl = slice(oc * 512, (oc + 1) * 512)
                        po = out_ps.tile([128, 512], f32, tag="po")
                        nc.tensor.matmul(
                            po[:],
                            wh_sb[:, ts],
                            b2_sb[:, osl],
                            start=True,
                            stop=True,
                        )
                        if oc % COPY_ACT_EVERY == COPY_ACT_EVERY - 1:
                            nc.scalar.copy(out=o_sb[:, osl], in_=po[:])
                        else:
                            nc.vector.tensor_copy(out=o_sb[:, osl], in_=po[:])
                if "outdma" not in ablate:
                    nc.sync.dma_start(out=out.ap()[ts, :], in_=o_sb[:])

    nc.compile()
    return nc


def _make_runner(nc, n_cores=N_CORES):
    import jax
    import numpy as np
    from jax.sharding import Mesh, NamedSharding, PartitionSpec
    from jax.experimental.shard_map import shard_map
    import concourse.mybir as mybir
    from concourse.bass2jax import (
        _bass_exec_p,
        install_neuronx_cc_hook,
        partition_id_tensor,
    )

    install_neuronx_cc_hook()
    partition_name = nc.partition_id_tensor.name if nc.partition_id_tensor else None
    in_names, out_names, out_avals = [], [], []
    for alloc in nc.m.functions[0].allocations:
        if not isinstance(alloc, mybir.MemoryLocationSet):
            continue
        name = alloc.memorylocations[0].name
        if alloc.kind == "ExternalInput":
            if name != partition_name:
                in_names.append(name)
        elif alloc.kind == "ExternalOutput":
            out_names.append(name)
            out_avals.append(
                jax.core.ShapedArray(
                    tuple(alloc.tensor_shape), mybir.dt.np(alloc.dtype)
                )
            )
    n_params = len(in_names)
    n_outs = len(out_avals)
    all_in_names = in_names + out_names + ([partition_name] if partition_name else [])

    def _body(*args):
        operands = list(args)
        if partition_name is not None:
            operands.append(partition_id_tensor())
        outs = _bass_exec_p.bind(
            *operands,
            out_avals=tuple(out_avals),
            in_names=tuple(all_in_names),
            out_names=tuple(out_names),
            lowering_input_output_aliases=(),
            sim_require_finite=True,
            sim_require_nnan=True,
            nc=nc,
        )
        return tuple(outs)

    devices = jax.devices()[:n_cores]
    mesh = Mesh(np.asarray(devices), ("core",))
    sharding = NamedSharding(mesh, PartitionSpec("core"))
    in_specs = (PartitionSpec("core"),) * (n_params + n_outs)
    out_specs = (PartitionSpec("core"),) * n_outs

    fn1 = jax.jit(
        shard_map(
            _body, mesh=mesh, in_specs=in_specs, out_specs=out_specs, check_rep=False
        ),
        donate_argnums=tuple(range(n_params, n_params + n_outs)),
        keep_unused=True,
    )

    def _chain_factory(k):
        def chain(*args):
            ins = list(args[:n_params])
            z = list(args[n_params:])
            for _ in range(k):
                z = list(_body(*ins, *z))
            return tuple(z)

        return jax.jit(
            shard_map(
                chain,
                mesh=mesh,
                in_specs=in_specs,
                out_specs=out_specs,
                check_rep=False,
            ),
            donate_argnums=tuple(range(n_params, n_params + n_outs)),
            keep_unused=True,
        )

    return {
        "fn1": fn1,
        "chain_factory": _chain_factory,
        "in_names": in_names,
        "out_names": out_names,
        "out_avals": out_avals,
        "mesh": mesh,
        "sharding": sharding,
        "n_params": n_params,
    }


def _get_runner():
    if "runner" not in _CACHE:
        nc = _build_nc()
        _CACHE["nc"] = nc
        _CACHE["runner"] = _make_runner(nc)
    return _CACHE["runner"]


def _prep_inputs(inputs, router_w, router_b, thr_w, thr_b, lora_A, lora_B):
    """Host-side staging: transposes + bf16 casts + sharding. Returns dict of
    global (concatenated along axis 0) arrays."""
    import ml_dtypes

    bf16 = ml_dtypes.bfloat16
    x = np.asarray(inputs, dtype=np.float32)
    xT = np.ascontiguousarray(x.T.astype(bf16)).reshape(DC, 128, N_TOKENS)
    # A2[d, (e, r)] with e-major columns
    A2 = (
        np.ascontiguousarray(np.asarray(lora_A, np.float32).reshape(ER, D_IN).T)
        .astype(bf16)
        .reshape(DC, 128, ER)
    )
    # W9[d, 0:8]=router, [d, 8]=thr, padded to 16 cols
    W9 = np.zeros((D_IN, 48), np.float32)
    W9[:, 0:8] = np.asarray(router_w, np.float32).T
    W9[:, 32] = np.asarray(thr_w, np.float32)[0]
    W9 = W9.astype(bf16).reshape(DC, 128, 48)
    B9 = np.zeros((1, 48), np.float32)
    B9[0, 0:8] = np.asarray(router_b, np.float32)
    B9[0, 32] = np.asarray(thr_b, np.float32)[0]
    B9 = B9.astype(bf16)
    # B2[(e, r), o], pre-scaled (the extra 16 undoes wsum16 = 16*wsum)
    B2 = np.ascontiguousarray(
        np.asarray(lora_B, np.float32).transpose(0, 2, 1).reshape(ER, D_OUT)
        * (SCALING * 16.0)
    ).astype(bf16)
    G9m = np.zeros((9, ER), np.float32)
    for e in range(E):
        G9m[e, e * R : (e + 1) * R] = 1.0
    G9m[8, :] = -1.0
    G9m = G9m.astype(bf16)
    per_core = {
        "A2": A2,
        "W9": W9,
        "B9": B9,
        "B2": B2,
        "G9": G9m,
    }
    arrays = {}
    for name in ("A2", "W9", "B9", "B2", "G9"):
        a = per_core[name]
        arrays[name] = np.broadcast_to(
            a, (N_CORES,) + a.shape
        ).reshape((N_CORES * a.shape[0],) + a.shape[1:])
    # xT shards: tokens split along the last axis -> per-core [DC, 128, NL]
    xT_shards = [
        np.ascontiguousarray(xT[:, :, c * NL : (c + 1) * NL]) for c in range(N_CORES)
    ]
    arrays["xT"] = np.concatenate(xT_shards, axis=0)
    return arrays


def kernel(inputs, router_w, router_b, thr_w, thr_b, lora_A, lora_B):
    import jax

    r = _get_runner()
    arrays = _prep_inputs(
        inputs, router_w, router_b, thr_w, thr_b, lora_A, lora_B
    )
    ins = [arrays[name] for name in r["in_names"]]
    zeros = [
        np.zeros((N_CORES * a.shape[0],) + a.shape[1:], a.dtype)
        for a in r["out_avals"]
    ]
    outs = r["fn1"](*ins, *zeros)
    out = np.asarray(jax.block_until_ready(outs[0]))
    return out.reshape(N_TOKENS, D_OUT).astype(np.float32)


# revision 38
# speedup vs baseline: 2.9140x; 1.4554x over previous
"""AdaMoLE forward on 8 Trainium2 NeuronCores (Bass/Tile), data-parallel over tokens.

Reference computation (per token n):
  logits = x @ router_w.T + router_b            [N, E]
  gate   = softmax(logits)                      [N, E]
  thr    = sigmoid(x @ thr_w.T + thr_b)/E       [N, 1]
  w      = relu(gate - thr); w /= max(sum(w), eps-guard)
  h      = einsum('nd,erd->ner', x, lora_A)
  out    = einsum('ner,eor->no', h * w[:, :, None], lora_B) * SCALING

Device mapping (per core, NL=1024 tokens):
  phase1: stream xT[d, tok] over 32 d-chunks; accumulate in PSUM:
          hT[er=128, tok] (lhsT=A2 chunks) and lgT[9, tok] (lhsT=[router|thr]T)
  gating: PE-transpose lgT -> [tok, 9]; softmax-free trick:
          w ∝ relu(exp(logits) - sum(exp)*thr)  (common positive scale cancels
          in the normalization), normalize by free-dim sum, transpose back,
          expand e->(e,r) rows via a 0/1 REP matmul -> wexpT[128, tok]
  phase2: whT = hT * wexpT;  out[tok, o] = whT.T @ B2  (B2 pre-scaled by SCALING)

All big operands are host-pre-transposed (and cast to bf16) so every DMA is
contiguous and the PE runs at full bf16 rate. Gating math runs in f32 on-chip;
PSUM accumulation is f32 throughout. Output is f32.
"""

import numpy as np

N_TOKENS = 8192
D_IN = 4096
D_OUT = 4096
E = 8
R = 16
ER = E * R  # 128
SCALING = 32.0 / 16.0
MAX_THRESHOLD = 1.0 / E
N_CORES = 8
NL = N_TOKENS // N_CORES  # tokens per core
DC = D_IN // 128  # 32 d-chunks
TCH = NL // 128  # 8 token chunks per core
OC = D_OUT // 512  # 8 output column chunks

_CACHE = {}

# tuning knobs (read at _build_nc time)
COPY_ACT_EVERY = 2  # 1 of every N phase-2 PSUM->SBUF copies goes to ScalarE
A2_DMA_SPLIT = 4  # weight DMA split count (lets phase-1 start earlier)
OUT_BF16 = True  # ship the output as bf16 and upcast on the host


def _build_nc(dbg=False, repeat=1, ablate=frozenset()):
    import concourse.mybir as mybir
    import concourse.tile as tile
    from concourse import bacc
    from concourse.masks import make_identity

    f32 = mybir.dt.float32
    bf16 = mybir.dt.bfloat16
    AF = mybir.ActivationFunctionType
    ALU = mybir.AluOpType

    nc = bacc.Bacc("TRN2", target_bir_lowering=False, debug=False)
    dbg_t = {}
    if dbg:
        dbg_t["lg"] = nc.declare_dram_parameter("dbg_lg", [9, NL], f32, isOutput=True)
        dbg_t["h"] = nc.declare_dram_parameter("dbg_h", [128, NL], f32, isOutput=True)
        dbg_t["wn"] = nc.declare_dram_parameter("dbg_wn", [E, NL], f32, isOutput=True)
        dbg_t["wh"] = nc.declare_dram_parameter("dbg_wh", [128, NL], f32, isOutput=True)

    xT = nc.declare_dram_parameter("xT", [DC, 128, NL], bf16, isOutput=False)
    A2 = nc.declare_dram_parameter("A2", [DC, 128, ER], bf16, isOutput=False)
    W9 = nc.declare_dram_parameter("W9", [DC, 128, 40], bf16, isOutput=False)
    B9 = nc.declare_dram_parameter("B9", [33, 1], f32, isOutput=False)
    B2 = nc.declare_dram_parameter("B2", [ER, D_OUT], bf16, isOutput=False)
    # G[0:8, (e,r)] = delta_{e,e'}; G[8, :] = -1  (expand-and-subtract matrix)
    G9 = nc.declare_dram_parameter("G9", [9, ER], bf16, isOutput=False)
    out_dt = bf16 if OUT_BF16 else f32
    out = nc.declare_dram_parameter("out", [NL, D_OUT], out_dt, isOutput=True)

    with tile.TileContext(nc) as tc:
        with (
            tc.tile_pool(name="const", bufs=1) as cpool,
            tc.tile_pool(name="xt", bufs=6) as xpool,
            tc.tile_pool(name="work", bufs=1) as wpool,
            tc.tile_pool(name="osb", bufs=3) as opool,
            tc.tile_pool(name="accps", bufs=2, space="PSUM") as acc_ps,
            tc.tile_pool(name="outps", bufs=4, space="PSUM") as out_ps,
        ):
          for _rep in range(repeat):
            # ---- constants / weights into SBUF ----
            a2_sb = cpool.tile([128, DC, ER], bf16)
            for q in range(A2_DMA_SPLIT):
                qs = slice(q * (DC // A2_DMA_SPLIT), (q + 1) * (DC // A2_DMA_SPLIT))
                nc.sync.dma_start(
                    out=a2_sb[:, qs, :],
                    in_=A2.ap()[qs].rearrange("a b c -> b a c"),
                )
            w9_sb = cpool.tile([128, DC, 40], bf16)
            nc.sync.dma_start(out=w9_sb[:], in_=W9.ap().rearrange("a b c -> b a c"))
            b9_sb = cpool.tile([33, 1], f32)
            nc.sync.dma_start(out=b9_sb[:], in_=B9.ap())
            b2_sb = cpool.tile([ER, D_OUT], bf16)
            nc.sync.dma_start(out=b2_sb[:], in_=B2.ap())
            g9_sb = cpool.tile([9, ER], bf16)
            nc.sync.dma_start(out=g9_sb[:], in_=G9.ap())
            ones_sb = cpool.tile([128, 512], bf16)
            nc.vector.memset(ones_sb[:], 1.0)

            # ---- phase 1: accumulate hT[128, NL] and lgT[9, NL] over d-chunks
            h_acc = acc_ps.tile([128, NL], f32, tag="acc")
            lg_acc = acc_ps.tile([128, NL], f32, tag="acc")
            for dc in range(DC):
                xt = xpool.tile([128, NL], bf16)
                nc.sync.dma_start(out=xt[:], in_=xT.ap()[dc])
                for g in range(NL // 512):
                    sl = slice(g * 512, (g + 1) * 512)
                    nc.tensor.matmul(
                        h_acc[:, sl],
                        a2_sb[:, dc, :],
                        xt[:, sl],
                        start=(dc == 0),
                        stop=(dc == DC - 1),
                    )
                    nc.tensor.matmul(
                        lg_acc[0:33, sl],
                        w9_sb[:, dc, 0:33],
                        xt[:, sl],
                        start=(dc == 0),
                        stop=(dc == DC - 1),
                    )

            # ---- gating, entirely in the [e, tok] domain (no transposes) ----
            ex_sb = wpool.tile([E, NL], bf16)
            thr_sb = wpool.tile([1, NL], f32)
            nc.scalar.activation(ex_sb[:], lg_acc[0:8, :], AF.Exp, bias=b9_sb[0:8, 0:1])
            nc.scalar.activation(thr_sb[:], lg_acc[32:33, :], AF.Sigmoid, bias=b9_sb[32:33, 0:1])
            # S[1, tok] = sum_e exp
            s_ps = acc_ps.tile([128, NL], f32, tag="acc")
            for g in range(NL // 512):
                sl = slice(g * 512, (g + 1) * 512)
                nc.tensor.matmul(
                    s_ps[0:1, sl],
                    ones_sb[0:8, 0:1],
                    ex_sb[:, sl],
                    start=True,
                    stop=True,
                )
            # nsthr = -(S * MAX_T) * thr
            nsthr_sb = wpool.tile([1, NL], bf16)
            nc.vector.scalar_tensor_tensor(
                out=nsthr_sb[:],
                in0=s_ps[0:1, :],
                scalar=-MAX_THRESHOLD,
                in1=thr_sb[:],
                op0=ALU.mult,
                op1=ALU.mult,
            )
            # P1[(e,r), tok] = exp_e - sthr  (delta-expand exp, add -sthr bcast)
            p1_ps = acc_ps.tile([128, NL], f32, tag="acc")
            for g in range(NL // 512):
                sl = slice(g * 512, (g + 1) * 512)
                nc.tensor.matmul(
                    p1_ps[:, sl], g9_sb[0:8, :], ex_sb[:, sl],
                    start=True, stop=False,
                )
                nc.tensor.matmul(
                    p1_ps[:, sl], ones_sb[0:1, 0:128], nsthr_sb[:, sl],
                    start=False, stop=True,
                )
            # wtexp = relu(P1)  (unnormalized weights, expanded to (e,r) rows)
            wtexp_sb = wpool.tile([128, NL], bf16)
            nc.vector.tensor_scalar_max(wtexp_sb[:], p1_ps[:], 0.0)
            # wsum16[1, tok] = sum over all 128 rows = 16 * sum_e w
            # (the 1/16 is folded into B2's host-side scale)
            ws_ps = acc_ps.tile([128, NL], f32, tag="acc")
            for g in range(NL // 512):
                sl = slice(g * 512, (g + 1) * 512)
                nc.tensor.matmul(
                    ws_ps[0:1, sl], ones_sb[:, 0:1], wtexp_sb[:, sl],
                    start=True, stop=True,
                )
            # guard exact-0 then reciprocal
            rcp_sb = wpool.tile([1, NL], bf16)
            nc.vector.tensor_scalar_max(ws_ps[0:1, :], ws_ps[0:1, :], 1e-30)
            with nc.allow_low_precision(reason="recip rounds to bf16 on write"):
                nc.vector.reciprocal(rcp_sb[:], ws_ps[0:1, :])
            # expand recip to all 128 rows
            rexp_ps = acc_ps.tile([128, NL], f32, tag="acc")
            for g in range(NL // 512):
                sl = slice(g * 512, (g + 1) * 512)
                nc.tensor.matmul(
                    rexp_ps[:, sl], ones_sb[0:1, 0:128], rcp_sb[:, sl],
                    start=True, stop=True,
                )
            # wn = wtexp * recip;  wh = h * wn
            wn_sb = wpool.tile([128, NL], f32)
            nc.vector.tensor_tensor(
                out=wn_sb[:], in0=rexp_ps[:], in1=wtexp_sb[:], op=ALU.mult
            )
            wh_sb = wpool.tile([128, NL], bf16)
            nc.vector.tensor_tensor(
                out=wh_sb[:], in0=h_acc[:], in1=wn_sb[:], op=ALU.mult
            )
            if dbg:
                h_sb = wpool.tile([128, NL], f32)
                nc.scalar.copy(out=h_sb[:], in_=h_acc[:])
                nc.sync.dma_start(out=dbg_t["h"].ap()[:, :], in_=h_sb[:])
                lg_sb = wpool.tile([9, NL], f32)
                nc.scalar.copy(out=lg_sb[:], in_=lg_acc[0:9, :])
                nc.sync.dma_start(out=dbg_t["lg"].ap()[:, :], in_=lg_sb[:])
                nc.sync.dma_start(out=dbg_t["wn"].ap()[:, :], in_=wn_sb[0:E, :])
                wh_f32 = wpool.tile([128, NL], f32)
                nc.vector.tensor_copy(out=wh_f32[:], in_=wh_sb[:])
                nc.sync.dma_start(out=dbg_t["wh"].ap()[:, :], in_=wh_f32[:])

            # ---- phase 2: out[tok, o] = whT.T @ B2 ----
            for t in range(TCH):
                ts = slice(t * 128, (t + 1) * 128)
                o_sb = opool.tile([128, D_OUT], out_dt, tag="osb")
                if "p2mm" in ablate:
                    nc.scalar.memzero(o_sb[:])
                else:
                    for oc in range(OC):
                        osl = slice(oc * 512, (oc + 1) * 512)
                        po = out_ps.tile([128, 512], f32, tag="po")
                        nc.tensor.matmul(
                            po[:],
                            wh_sb[:, ts],
                            b2_sb[:, osl],
                            start=True,
                            stop=True,
                        )
                        if oc % COPY_ACT_EVERY == COPY_ACT_EVERY - 1:
                            nc.scalar.copy(out=o_sb[:, osl], in_=po[:])
                        else:
                            nc.vector.tensor_copy(out=o_sb[:, osl], in_=po[:])
                if "outdma" not in ablate:
                    nc.sync.dma_start(out=out.ap()[ts, :], in_=o_sb[:])

    nc.compile()
    return nc


def _make_runner(nc, n_cores=N_CORES):
    import jax
    import numpy as np
    from jax.sharding import Mesh, NamedSharding, PartitionSpec
    from jax.experimental.shard_map import shard_map
    import concourse.mybir as mybir
    from concourse.bass2jax import (
        _bass_exec_p,
        install_neuronx_cc_hook,
        partition_id_tensor,
    )

    install_neuronx_cc_hook()
    partition_name = nc.partition_id_tensor.name if nc.partition_id_tensor else None
    in_names, out_names, out_avals = [], [], []
    for alloc in nc.m.functions[0].allocations:
        if not isinstance(alloc, mybir.MemoryLocationSet):
            continue
        name = alloc.memorylocations[0].name
        if alloc.kind == "ExternalInput":
            if name != partition_name:
                in_names.append(name)
        elif alloc.kind == "ExternalOutput":
            out_names.append(name)
            out_avals.append(
                jax.core.ShapedArray(
                    tuple(alloc.tensor_shape), mybir.dt.np(alloc.dtype)
                )
            )
    n_params = len(in_names)
    n_outs = len(out_avals)
    all_in_names = in_names + out_names + ([partition_name] if partition_name else [])

    def _body(*args):
        operands = list(args)
        if partition_name is not None:
            operands.append(partition_id_tensor())
        outs = _bass_exec_p.bind(
            *operands,
            out_avals=tuple(out_avals),
            in_names=tuple(all_in_names),
            out_names=tuple(out_names),
            lowering_input_output_aliases=(),
            sim_require_finite=True,
            sim_require_nnan=True,
            nc=nc,
        )
        return tuple(outs)

    devices = jax.devices()[:n_cores]
    mesh = Mesh(np.asarray(devices), ("core",))
    sharding = NamedSharding(mesh, PartitionSpec("core"))
    in_specs = (PartitionSpec("core"),) * (n_params + n_outs)
    out_specs = (PartitionSpec("core"),) * n_outs

    fn1 = jax.jit(
        shard_map(
            _body, mesh=mesh, in_specs=in_specs, out_specs=out_specs, check_rep=False
        ),
        donate_argnums=tuple(range(n_params, n_params + n_outs)),
        keep_unused=True,
    )

    def _chain_factory(k):
        def chain(*args):
            ins = list(args[:n_params])
            z = list(args[n_params:])
            for _ in range(k):
                z = list(_body(*ins, *z))
            return tuple(z)

        return jax.jit(
            shard_map(
                chain,
                mesh=mesh,
                in_specs=in_specs,
                out_specs=out_specs,
                check_rep=False,
            ),
            donate_argnums=tuple(range(n_params, n_params + n_outs)),
            keep_unused=True,
        )

    return {
        "fn1": fn1,
        "chain_factory": _chain_factory,
        "in_names": in_names,
        "out_names": out_names,
        "out_avals": out_avals,
        "mesh": mesh,
        "sharding": sharding,
        "n_params": n_params,
    }


def _get_runner():
    if "runner" not in _CACHE:
        nc = _build_nc()
        _CACHE["nc"] = nc
        _CACHE["runner"] = _make_runner(nc)
    return _CACHE["runner"]


def _prep_inputs(inputs, router_w, router_b, thr_w, thr_b, lora_A, lora_B):
    """Host-side staging: transposes + bf16 casts + sharding. Returns dict of
    global (concatenated along axis 0) arrays."""
    import ml_dtypes

    bf16 = ml_dtypes.bfloat16
    x = np.asarray(inputs, dtype=np.float32)
    xT = np.ascontiguousarray(x.T.astype(bf16)).reshape(DC, 128, N_TOKENS)
    # A2[d, (e, r)] with e-major columns
    A2 = (
        np.ascontiguousarray(np.asarray(lora_A, np.float32).reshape(ER, D_IN).T)
        .astype(bf16)
        .reshape(DC, 128, ER)
    )
    # W9[d, 0:8]=router, [d, 8]=thr, padded to 16 cols
    W9 = np.zeros((D_IN, 40), np.float32)
    W9[:, 0:8] = np.asarray(router_w, np.float32).T
    W9[:, 32] = np.asarray(thr_w, np.float32)[0]
    W9 = W9.astype(bf16).reshape(DC, 128, 40)
    B9 = np.zeros((33, 1), np.float32)
    B9[0:8, 0] = np.asarray(router_b, np.float32)
    B9[32, 0] = np.asarray(thr_b, np.float32)[0]
    # B2[(e, r), o], pre-scaled (the extra 16 undoes wsum16 = 16*wsum)
    B2 = np.ascontiguousarray(
        np.asarray(lora_B, np.float32).transpose(0, 2, 1).reshape(ER, D_OUT)
        * (SCALING * 16.0)
    ).astype(bf16)
    G9m = np.zeros((9, ER), np.float32)
    for e in range(E):
        G9m[e, e * R : (e + 1) * R] = 1.0
    G9m[8, :] = -1.0
    G9m = G9m.astype(bf16)
    per_core = {
        "A2": A2,
        "W9": W9,
        "B9": B9,
        "B2": B2,
        "G9": G9m,
    }
    arrays = {}
    for name in ("A2", "W9", "B9", "B2", "G9"):
        a = per_core[name]
        arrays[name] = np.broadcast_to(
            a, (N_CORES,) + a.shape
        ).reshape((N_CORES * a.shape[0],) + a.shape[1:])
    # xT shards: tokens split along the last axis -> per-core [DC, 128, NL]
    xT_shards = [
        np.ascontiguousarray(xT[:, :, c * NL : (c + 1) * NL]) for c in range(N_CORES)
    ]
    arrays["xT"] = np.concatenate(xT_shards, axis=0)
    return arrays


def kernel(inputs, router_w, router_b, thr_w, thr_b, lora_A, lora_B):
    import jax

    r = _get_runner()
    arrays = _prep_inputs(
        inputs, router_w, router_b, thr_w, thr_b, lora_A, lora_B
    )
    ins = [arrays[name] for name in r["in_names"]]
    zeros = [
        np.zeros((N_CORES * a.shape[0],) + a.shape[1:], a.dtype)
        for a in r["out_avals"]
    ]
    outs = r["fn1"](*ins, *zeros)
    out = np.asarray(jax.block_until_ready(outs[0]))
    return out.reshape(N_TOKENS, D_OUT).astype(np.float32)
